# revision 67
# baseline (speedup 1.0000x reference)
"""Trainium2 Bass kernel for a 2-layer GATv2 (nn_GAT_40372692582770).

Gather-free, PE-centric design (no custom GPSIMD ucode needed):
  - Nodes partitioned by dst range across 8 cores; edges (+self loops)
    routed to the dst owner, sorted by dst, grouped into 128-dst strips,
    padded per strip to a uniform B 128-edge blocks (shared program).
  - Host ships, per layer, per-edge feature columns (the "halo exchange"
    materialized host-side, since the graph is static):
      xsdT [128, S]       = [x[src_e]; x[dst_e]] columns, stacked  (fp16)
      xe   [128, NBLK*68] = x[src_e] rows + ones col, partition-
                            major for 4KB-run DMA descriptors       (fp16)
      dstloc2 [128, NBLK*2] = dst-in-strip ids, duplicated pairs so the
                            DVE one-hot compare runs in 2x_1P mode
  - Device per chunk:
      z[c,e]  = Wsd^T xsd                     (PE, k=128, PSUM)
      L2      = prelu(z + bz, 0.2)            (ACT, fp16)
      e[e,h]  = L2_blk^T @ A                  (PE, per 128-edge block)
      w       = exp(e - 2)                    (ACT, fp16)
      oh      = (dstloc2 == iota)             (DVE 2x_1P, paired APs)
      xew_h   = xe * w2_h                     (DVE 2x_1P, paired APs)
      GT[j,(h,s)] += xew_h_blk^T @ oh_blk     (PE, per strip, PSUM)
      strip:  out[s, 130] = GT_h0^T @ R2_0 + GT_h1^T @ R2_1  (PE)
              cols = [num_h0 | num_h1 | 2*den_0 | 2*den_1]
  - Finalize (3 interleaved blocks, overlapped with chunk compute):
    alpha-normalize, head-mean, +bias, gelu -> out_raw fp32 + out_act
    fp16, both partition-major (host un-permutes). Host glue
    (concat/transpose/np-take only) between layers.

One program serves both layers (weights are inputs); compiled once.
"""
import os
import sys
import time

sys.path.insert(0, "/opt/trn_rl_repo")

import numpy as np

import concourse.bass as bass
import concourse.mybir as mybir
import concourse.tile as tile
from concourse import bacc
from concourse.bass_utils import run_bass_kernel_spmd

class Cfg:
    N = 100000
    D = 64
    H = 2
    C = 64
    NCORES = 8
    SPC = 3            # strips per chunk
    ESHIFT = -2.0      # exp bias

    @property
    def RN(self):
        return self.N // self.NCORES

    @property
    def NSTRIP(self):
        return (self.RN + 127) // 128

    @property
    def NSTRIP_PAD(self):
        return ((self.NSTRIP + self.SPC - 1) // self.SPC) * self.SPC

    @property
    def HC(self):
        return self.H * self.C


CFG = Cfg()
FP16 = mybir.dt.float16
FP32 = mybir.dt.float32
AF = mybir.ActivationFunctionType
ALU = mybir.AluOpType


# ------------------------------------------------------------- host prep
def _prep_edges(cfg, src, dst):
    """Route edges to dst-owner cores; LPT-balance local nodes across strips
    (so max strip load ~= mean and B shrinks); sort edges by assigned slot;
    pad strips to B blocks. Returns (B, per-core (srcids, dstloc) in slot
    order with pad slots src=-1 dstloc=255, per-core position permutation
    pos_of_node [RN]). Strip of a node = pos>>7, in-strip loc = pos&127."""
    RN = cfg.RN
    NSA = cfg.NSTRIP_PAD            # strips available for assignment
    core = dst // RN
    per_core = []
    maxblk = 1
    for c in range(cfg.NCORES):
        sel = np.flatnonzero(core == c)
        d = (dst[sel] - c * RN).astype(np.int64)
        s = src[sel].astype(np.int64)
        deg = np.bincount(d, minlength=RN)
        # LPT: place nodes (desc degree) on the least-loaded non-full strip
        order = np.argsort(-deg, kind="stable")
        loads = np.zeros(NSA, np.int64)
        fill = np.zeros(NSA, np.int64)
        pos_of_node = np.empty(RN, np.int64)
        for n in order:
            stp = int(np.argmin(np.where(fill < 128, loads, 1 << 40)))
            pos_of_node[n] = stp * 128 + fill[stp]
            fill[stp] += 1
            loads[stp] += deg[n]
        p = pos_of_node[d]
        o = np.argsort(p, kind="stable")
        p, s = p[o], s[o]
        cnt = np.bincount(p >> 7, minlength=NSA)
        maxblk = max(maxblk, int((cnt.max() + 127) // 128))
        per_core.append((s, p, cnt, pos_of_node))
    B = maxblk
    nslot = NSA * B * 128
    out = []
    perms = []
    for c in range(cfg.NCORES):
        s, p, cnt, pos_of_node = per_core[c]
        sids = np.full(nslot, -1, np.int64)
        dloc = np.full(nslot, 255, np.int64)
        pos = 0
        for st in range(NSA):
            k = int(cnt[st])
            base = st * B * 128
            sids[base:base + k] = s[pos:pos + k]
            dloc[base:base + k] = p[pos:pos + k] & 127
            pos += k
        out.append((sids, dloc))
        perms.append(pos_of_node)
    return B, out, perms


def _prep_layer_weights(cfg, Wl, bl, Wr, br, att, bias):
    D, H, C = cfg.D, cfg.H, cfg.C
    HC = cfg.HC
    # Wsd [128, 128]: stacked [Wl; Wr] so z = Wsd^T [xs; xd]
    wsd = np.vstack([Wl, Wr])
    biasZ = (bl + br).astype(np.float32)[:, None]
    # A [128, 2]: att dot (block-diagonal per head); prelu handles the slope
    A = np.zeros((HC, H), np.float64)
    for h in range(H):
        A[h * C:(h + 1) * C, h] = att[h]
    # R2_h [66, 130]: second-level aggregation weights
    # den column scaled 2x so finalize's head-mean 0.5 factor is free
    R2 = np.zeros((H, 66, 130), np.float64)
    for h in range(H):
        R2[h, :D, h * C:(h + 1) * C] = Wl[:, h * C:(h + 1) * C]
        R2[h, D, h * C:(h + 1) * C] = bl[h * C:(h + 1) * C]
        R2[h, D, HC + h] = 2.0          # denominator column
    biasF = np.tile(bias.astype(np.float32)[None, :], (128, 1))
    return {
        "wsd": wsd.astype(np.float16), "biasZ": biasZ,
        "Amat": A.astype(np.float16),
        "R2_0": R2[0].astype(np.float16), "R2_1": R2[1].astype(np.float16),
        "biasF": biasF,
    }


# --------------------------------------------------------- program build
def build_program(cfg, B, last=False):
    D, H, C = cfg.D, cfg.H, cfg.C
    HC = cfg.HC
    NBLK = cfg.NSTRIP_PAD * B
    NCHUNK = cfg.NSTRIP_PAD // cfg.SPC
    CB = cfg.SPC * B                   # blocks per chunk
    CS = CB * 128                      # slots per chunk
    S = NBLK * 128
    RROW = HC + 2                      # strip psum row [num128 | d0 d1]
    NS = cfg.NSTRIP_PAD

    nc = bacc.Bacc("TRN2", target_bir_lowering=False, debug=False,
                   num_devices=cfg.NCORES)

    xsdT = nc.declare_dram_parameter("xsdT", [2 * D, S], FP16, isOutput=False)
    xe = nc.declare_dram_parameter("xe", [128, NBLK * 68], FP16, isOutput=False)
    dstloc2 = nc.declare_dram_parameter("dstloc2", [128, NBLK * 2], FP16,
                                        isOutput=False)
    iotaF = nc.declare_dram_parameter("iotaF", [128, 128], FP16, isOutput=False)
    wsd = nc.declare_dram_parameter("wsd", [2 * D, HC], FP16, isOutput=False)
    biasZ = nc.declare_dram_parameter("biasZ", [HC, 1], FP32, isOutput=False)
    Amat = nc.declare_dram_parameter("Amat", [HC, H], FP16, isOutput=False)
    R2_0 = nc.declare_dram_parameter("R2_0", [66, 130], FP16, isOutput=False)
    R2_1 = nc.declare_dram_parameter("R2_1", [66, 130], FP16, isOutput=False)
    biasF = nc.declare_dram_parameter("biasF", [128, C], FP32, isOutput=False)
    out_raw = nc.declare_dram_parameter("out_raw", [128, NS * C], FP32,
                                        isOutput=True)
    out_act = nc.declare_dram_parameter("out_act", [128, NS * C], FP16,
                                        isOutput=True)

    with tile.TileContext(nc) as tc:
        with (
            tc.tile_pool(name="const", bufs=1) as cpool,
            tc.tile_pool(name="stash", bufs=1) as stpool,
        ):
            wsd_t = cpool.tile([2 * D, HC], FP16)
            nc.sync.dma_start(out=wsd_t[:], in_=wsd[:, :])
            bz_t = cpool.tile([HC, 1], FP32)
            nc.sync.dma_start(out=bz_t[:], in_=biasZ[:, :])
            A_t = cpool.tile([HC, H], FP16)
            nc.sync.dma_start(out=A_t[:], in_=Amat[:, :])
            r2_t = [cpool.tile([66, 130], FP16, tag=f"r2{h}", name=f"r2{h}") for h in range(H)]
            nc.sync.dma_start(out=r2_t[0][:], in_=R2_0[:, :])
            nc.sync.dma_start(out=r2_t[1][:], in_=R2_1[:, :])
            ebias_t = cpool.tile([128, 1], FP32)
            nc.vector.memset(ebias_t[:], cfg.ESHIFT)
            dl2_t = cpool.tile([128, NBLK * 2], FP16)
            dl2v = dl2_t[:].rearrange("p (n k) -> p n k", k=2)
            iota_t = cpool.tile([128, 128], FP16)

            bias_t = cpool.tile([128, C], FP32)
            nc.sync.dma_start(out=bias_t[:], in_=biasF[:, :])

            stash = stpool.tile([128, NS * (HC + 2)], FP32)
            sv = stash[:].rearrange("p (s w) -> p s w", w=HC + 2)
            tmean = stpool.tile([128, NS * C], FP32, tag="tmean")
            tmv = tmean[:].rearrange("p (s c) -> p s c", c=C)
            outg = stpool.tile([128, NS * C], FP16, tag="outg")
            ogvv = outg[:].rearrange("p (s c) -> p s c", c=C)

            with (
                tc.tile_pool(name="eg", bufs=2) as egpool,
                tc.tile_pool(name="ez", bufs=2) as ezpool,
                tc.tile_pool(name="esm", bufs=3) as smpool,
                tc.tile_pool(name="fin", bufs=1) as fpool,
                tc.tile_pool(name="zps", bufs=2, space="PSUM") as zpspool,
                tc.tile_pool(name="eps", bufs=2, space="PSUM") as epspool,
                tc.tile_pool(name="gps", bufs=2, space="PSUM") as gpspool,
                tc.tile_pool(name="sps", bufs=2, space="PSUM") as spspool,
            ):
                for ch in range(NCHUNK):
                    c0 = ch * CS
                    b0 = ch * CB
                    xsd_t = egpool.tile([2 * D, CS], FP16, tag="xsd")
                    nc.sync.dma_start(out=xsd_t[:], in_=xsdT[:, c0:c0 + CS])
                    xe_t = egpool.tile([128, CB * 68], FP16, tag="xe")
                    nc.sync.dma_start(
                        out=xe_t[:], in_=xe[:, b0 * 68:(b0 + CB) * 68])
                    if ch == 0:
                        # deferred const loads: after chunk-0 data so the
                        # first zT matmul isn't queued behind 1.2MB of dstloc
                        nc.sync.dma_start(out=dl2_t[:], in_=dstloc2[:, :])
                        nc.sync.dma_start(out=iota_t[:], in_=iotaF[:, :])
                    xev = xe_t[:].rearrange("p (b w) -> p b w", w=68)
                    # oh via paired compare: every operand's innermost AP dim
                    # is unit-stride pairs -> DVE 2x_1P mode
                    oh_t = egpool.tile([128, CB * 128], FP16, tag="oh")
                    ohv = oh_t[:].rearrange("p (b s) -> p b s", s=128)
                    ohv4 = oh_t[:].rearrange("p (b s k) -> p b s k", s=64, k=2)
                    iov4 = iota_t[:].rearrange("p (s k) -> p s k", k=2)
                    nc.vector.tensor_tensor(
                        out=ohv4[:, :, :, :],
                        in0=dl2v[:, b0:b0 + CB, :].unsqueeze(2).to_broadcast(
                            [128, CB, 64, 2]),
                        in1=iov4[:].unsqueeze(1).to_broadcast([128, CB, 64, 2]),
                        op=ALU.is_equal)

                    # z feature-major in groups of 512 edges; prelu -> L2
                    L2 = ezpool.tile([128, CS], FP16, tag="L2")
                    ngrp = (CS + 511) // 512
                    for g in range(ngrp):
                        g0 = g * 512
                        gw = min(512, CS - g0)
                        zp = zpspool.tile([128, 512], FP32, tag="zp")
                        nc.tensor.matmul(zp[:, :gw], lhsT=wsd_t[:],
                                         rhs=xsd_t[:, g0:g0 + gw],
                                         start=True, stop=True)
                        nc.scalar.activation(out=L2[:, g0:g0 + gw],
                                             in_=zp[:, :gw], func=AF.Prelu,
                                             bias=bz_t[:], alpha=0.2)

                    # e-dot per block -> e psum [128, 2*CB]
                    ep = epspool.tile([128, 2 * CB], FP32, tag="ep")
                    for b in range(CB):
                        nc.tensor.matmul(ep[:, 2 * b:2 * b + 2],
                                         lhsT=L2[:, b * 128:(b + 1) * 128],
                                         rhs=A_t[:], start=True, stop=True)
                    w_t = smpool.tile([128, 2 * CB], FP16, tag="w")
                    wv = w_t[:].rearrange("p (b k) -> p b k", k=2)
                    nc.scalar.activation(out=w_t[:], in_=ep[:], func=AF.Exp,
                                         bias=ebias_t[:])

                    # w2: duplicate w into adjacent pairs for 2x_1P broadcast
                    w2 = smpool.tile([128, CB * 2 * 2], FP16, tag="w2")
                    w2v = w2[:].rearrange("p (b h k) -> p b h k", h=2, k=2)
                    nc.vector.tensor_copy(
                        w2v[:, :, :, :],
                        wv[:, :, :].unsqueeze(3).to_broadcast([128, CB, 2, 2]))
                    # xew_h = xe * w_h  [128, (b, h, 68)], paired -> 2x_1P
                    xew = ezpool.tile([128, CB * 2 * 68], FP16, tag="xew")
                    xwv = xew[:].rearrange("p (b h w) -> p b h w", h=2, w=68)
                    xwv4 = xew[:].rearrange("p (b h w k) -> p b h w k", h=2,
                                            w=34, k=2)
                    xev4 = xe_t[:].rearrange("p (b w k) -> p b w k", w=34, k=2)
                    for h in range(H):
                        nc.vector.tensor_tensor(
                            out=xwv4[:, :, h, :, :],
                            in0=xev4[:, :, :, :],
                            in1=w2v[:, :, h, :].unsqueeze(2).to_broadcast(
                                [128, CB, 34, 2]),
                            op=ALU.mult)

                    # GT per strip + strip-level matmuls
                    for s3 in range(cfg.SPC):
                        st = ch * cfg.SPC + s3
                        gt = gpspool.tile([66, 256], FP32, tag="gt")
                        for h in range(H):
                            for b in range(B):
                                blk = s3 * B + b
                                nc.tensor.matmul(
                                    gt[:, h * 128:(h + 1) * 128],
                                    lhsT=xwv[:, blk, h, 0:66],
                                    rhs=ohv[:, blk, :],
                                    start=(b == 0), stop=(b == B - 1))
                        gts = smpool.tile([66, 256], FP16, tag="gts")
                        if st % 3 == 0 and st < 60:
                            nc.vector.tensor_copy(gts[:], gt[:])
                        else:
                            nc.scalar.copy(gts[:], gt[:])
                        sp = spspool.tile([128, HC + 2], FP32, tag="sp")
                        nc.tensor.matmul(sp[:], lhsT=gts[:, 0:128], rhs=r2_t[0][:],
                                         start=True, stop=False)
                        nc.tensor.matmul(sp[:], lhsT=gts[:, 128:256], rhs=r2_t[1][:],
                                         start=False, stop=True)
                        dst_sl = stash[:, st * (HC + 2):(st + 1) * (HC + 2)]
                        if s3 % 2 == 0:
                            nc.vector.tensor_copy(dst_sl, sp[:])
                        else:
                            nc.scalar.copy(dst_sl, sp[:])

                    # --- interleaved finalize: blocks (40, 40, rest), with
                    # per-block output DMA so the kernel tail is short ---
                    FIN = {9: (0, 30), 19: (30, 30), 29: (60, 30),
                           NCHUNK - 1: (90, NS - 90)}
                    if ch in FIN:
                        f0, FB = FIN[ch]
                        svc = sv[:, f0:f0 + FB, :]
                        rec = fpool.tile([128, FB * 2], FP32, tag="rec")
                        recv = rec[:].rearrange("p (s k) -> p s k", k=2)
                        nc.vector.reciprocal(out=recv[:, :, :],
                                             in_=svc[:, :, HC:HC + 2])
                        tm = tmv[:, f0:f0 + FB, :]
                        nc.vector.tensor_tensor(
                            out=tm, in0=svc[:, :, 0:C],
                            in1=recv[:, :, 0:1].to_broadcast([128, FB, C]),
                            op=ALU.mult)
                        t2 = fpool.tile([128, FB * C], FP32, tag="t2")
                        t2v = t2[:].rearrange("p (s c) -> p s c", c=C)
                        nc.vector.tensor_tensor(
                            out=t2v[:, :, :], in0=svc[:, :, C:2 * C],
                            in1=recv[:, :, 1:2].to_broadcast([128, FB, C]),
                            op=ALU.mult)
                        nc.vector.tensor_tensor(out=tm, in0=tm,
                                                in1=t2v[:, :, :], op=ALU.add)
                        nc.vector.tensor_tensor(
                            out=tm, in0=tm,
                            in1=bias_t[:].unsqueeze(1).to_broadcast(
                                [128, FB, C]),
                            op=ALU.add)
                        e1 = f0 + FB
                        if not last:
                            # gelu(x) = x*sigmoid(1.5957*x*(1+0.044715x^2))
                            # (only needed to feed the next layer)
                            cub = fpool.tile([128, FB * C], FP32, tag="cub")
                            cv = cub[:].rearrange("p (s c) -> p s c", c=C)
                            nc.scalar.activation(out=cv[:, :, :], in_=tm,
                                                 func=AF.Square,
                                                 scale=0.2114592836514295)
                            nc.vector.scalar_tensor_tensor(
                                out=cv[:, :, :], in0=cv[:, :, :], scalar=1.0,
                                in1=tm, op0=ALU.add, op1=ALU.mult)
                            nc.scalar.activation(out=cv[:, :, :],
                                                 in_=cv[:, :, :],
                                                 func=AF.Sigmoid,
                                                 scale=1.5957691216057308)
                            nc.vector.tensor_tensor(
                                out=ogvv[:, f0:f0 + FB, :], in0=cv[:, :, :],
                                in1=tm, op=ALU.mult)
                            nc.sync.dma_start(out=out_act[:, f0 * C:e1 * C],
                                              in_=outg[:, f0 * C:e1 * C])
                        nc.sync.dma_start(out=out_raw[:, f0 * C:e1 * C],
                                          in_=tmean[:, f0 * C:e1 * C])

    nc.compile()
    return nc




# ----------------------------------------------------- persistent runner
class Runner:
    """Jit-compiled SPMD callable with reusable device inputs (no donation)."""

    def __init__(self, nc, n_cores):
        import jax
        import concourse.mybir as mb
        from concourse import bass2jax
        from jax.experimental.shard_map import shard_map
        from jax.sharding import Mesh, PartitionSpec
        bass2jax.install_neuronx_cc_hook()
        self.nc = nc
        self.n_cores = n_cores
        in_names, out_names, out_avals, zero_outs = [], [], [], []
        for alloc in nc.m.functions[0].allocations:
            if not isinstance(alloc, mb.MemoryLocationSet):
                continue
            name = alloc.memorylocations[0].name
            if alloc.kind == "ExternalInput":
                in_names.append(name)
            elif alloc.kind == "ExternalOutput":
                out_names.append(name)
                shape = tuple(alloc.tensor_shape)
                dtype = mb.dt.np(alloc.dtype)
                out_avals.append(jax.core.ShapedArray(shape, dtype))
                zero_outs.append(np.zeros(shape, dtype))
        pt = nc.partition_id_tensor
        self.pname = pt.name if pt else None
        if self.pname in in_names:
            in_names.remove(self.pname)
        self.in_names = list(in_names)
        self.out_names = list(out_names)
        self.out_avals = out_avals
        self.zero_outs = zero_outs
        all_in = list(in_names) + list(out_names)
        if self.pname:
            all_in.append(self.pname)

        def _body(*args):
            operands = list(args)
            if self.pname:
                operands.append(bass2jax.partition_id_tensor())
            outs = bass2jax._bass_exec_p.bind(
                *operands,
                out_avals=tuple(out_avals),
                in_names=tuple(all_in),
                out_names=tuple(out_names),
                lowering_input_output_aliases=(),
                sim_require_finite=True,
                sim_require_nnan=True,
                nc=nc,
            )
            return tuple(outs)

        devices = jax.devices()[:n_cores]
        self.mesh = Mesh(np.asarray(devices), ("core",))
        np_in = (PartitionSpec("core"),) * (len(in_names) + len(out_names))
        np_out = (PartitionSpec("core"),) * len(out_names)
        self.fn = jax.jit(shard_map(_body, mesh=self.mesh, in_specs=np_in,
                                    out_specs=np_out, check_rep=False),
                          keep_unused=True)

    def put(self, in_maps):
        """Concat per-core inputs and move to device. Returns arg list."""
        import jax
        from jax.sharding import NamedSharding, PartitionSpec
        sh = NamedSharding(self.mesh, PartitionSpec("core"))
        args = []
        for name in self.in_names:
            cat = np.concatenate([np.asarray(m[name]) for m in in_maps], axis=0)
            args.append(jax.device_put(cat, sh))
        for z in self.zero_outs:
            zz = np.zeros((self.n_cores * z.shape[0], *z.shape[1:]), z.dtype)
            args.append(jax.device_put(zz, sh))
        return args

    def run(self, args):
        return self.fn(*args)

    def results(self, out_arrs):
        res = []
        for c in range(self.n_cores):
            res.append({
                name: np.asarray(out_arrs[i]).reshape(
                    self.n_cores, *self.out_avals[i].shape)[c]
                for i, name in enumerate(self.out_names)})
        return res


# ------------------------------------------------------------- kernel()
_CACHE = {}
_RUNNERS = {}
LAST_ARGS = None
LAST_RUNNERS = None
LAST_LAUNCH_NS = None


def build_in_map(cfg, cur, slots, dstid, dlocP, nodepos, lw, iotaF, c):
    S = len(slots[c][0])
    sids, _ = slots[c]
    pad = sids < 0
    xs = cur[np.where(pad, 0, sids)]
    xs[pad] = 0
    dg = dstid[c]
    padd = dg < 0
    dn = nodepos[c][np.where(padd, 0, dg)]       # position -> local node id
    xd = cur[c * cfg.RN + np.where(padd | (dn < 0), 0, dn)]
    xd[padd] = 0
    xsdT = np.empty((2 * cfg.D, S), np.float16)
    xsdT[:cfg.D] = xs.T
    xsdT[cfg.D:] = xd.T
    xe_arr = np.zeros((S, 68), np.float16)
    xe_arr[:, :cfg.D] = xs
    xe_arr[:, cfg.D] = (~pad).astype(np.float16)
    # partition-major layout: [128, NBLK*68], row p = slots p, 128+p, ...
    NBLK = S // 128
    xe_arr = np.ascontiguousarray(
        xe_arr.reshape(NBLK, 128, 68).transpose(1, 0, 2).reshape(128, NBLK * 68))
    return {
        "xsdT": xsdT, "xe": xe_arr, "dstloc2": dlocP[c], "iotaF": iotaF,
        "wsd": lw["wsd"], "biasZ": lw["biasZ"], "Amat": lw["Amat"],
        "R2_0": lw["R2_0"], "R2_1": lw["R2_1"], "biasF": lw["biasF"],
    }


def prep_all(cfg, src, dst):
    B, slots, perms = _prep_edges(cfg, src, dst)
    S = cfg.NSTRIP_PAD * B * 128
    NBLK = cfg.NSTRIP_PAD * B
    dstid = []
    dlocP = []
    nodepos = []
    for c in range(cfg.NCORES):
        sids, dloc = slots[c]
        strip = (np.arange(S) // (B * 128))
        dstid.append(np.where(sids < 0, -1, strip * 128 + dloc))
        dp = dloc.reshape(NBLK, 128).T.astype(np.float16)   # [128, NBLK]
        dlocP.append(np.ascontiguousarray(
            np.repeat(dp, 2, axis=1)))                      # [128, NBLK*2]
        nop = np.full(cfg.NSTRIP_PAD * 128, -1, np.int64)
        nop[perms[c]] = np.arange(cfg.RN)
        nodepos.append(nop)
    return B, slots, dstid, dlocP, nodepos, perms


def kernel(embeded_nodes_features, edges_connectivity, Wl, bl, Wr, br, att, bias):
    global LAST_LAUNCH_NS
    cfg = CFG
    x = np.asarray(embeded_nodes_features, np.float32)
    ec = np.asarray(edges_connectivity)
    src = np.concatenate([ec[0], np.arange(cfg.N, dtype=ec.dtype)]).astype(np.int64)
    dst = np.concatenate([ec[1], np.arange(cfg.N, dtype=ec.dtype)]).astype(np.int64)
    Wl = np.asarray(Wl, np.float32)
    bl = np.asarray(bl, np.float32)
    Wr = np.asarray(Wr, np.float32)
    br = np.asarray(br, np.float32)
    att = np.asarray(att, np.float32)
    bias = np.asarray(bias, np.float32)
    L = Wl.shape[0]

    B, slots, dstid, dlocP, nodepos, perms = prep_all(cfg, src, dst)
    lws = [_prep_layer_weights(cfg, Wl[i], bl[i], Wr[i], br[i], att[i], bias[i])
           for i in range(L)]
    iotaF = np.tile(np.arange(128, dtype=np.float16)[None, :], (128, 1))

    cur = x.astype(np.float16)
    out_full = None
    _launch_ns = []
    _args_hist = []
    _runners_hist = []
    for i in range(L):
        lw = lws[i]
        in_maps = [build_in_map(cfg, cur, slots, dstid, dlocP, nodepos, lw,
                                iotaF, c)
                   for c in range(cfg.NCORES)]
        key = (B, i == L - 1)
        if key not in _CACHE:
            _CACHE[key] = build_program(cfg, B, last=(i == L - 1))
        if key not in _RUNNERS:
            _RUNNERS[key] = Runner(_CACHE[key], cfg.NCORES)
        runner = _RUNNERS[key]
        _runners_hist.append(runner)
        args = runner.put(in_maps)
        _args_hist.append(args)
        t0 = time.time()
        outs = runner.run(args)
        import jax
        jax.block_until_ready(outs)
        _launch_ns.append(int((time.time() - t0) * 1e9))
        res = runner.results(outs)
        NSr = cfg.NSTRIP_PAD

        def unperm(a, c):
            # [128, NSr*C] partition-major -> per-position rows -> node order
            posarr = a.reshape(128, NSr, cfg.C).transpose(1, 0, 2).reshape(
                NSr * 128, cfg.C)
            return posarr[perms[c]]
        raw = np.concatenate(
            [unperm(res[c]["out_raw"], c) for c in range(cfg.NCORES)], axis=0)
        actv = np.concatenate(
            [unperm(res[c]["out_act"], c) for c in range(cfg.NCORES)], axis=0)
        out_full = raw
        cur = actv
    LAST_LAUNCH_NS = _launch_ns
    global LAST_ARGS, LAST_RUNNERS
    LAST_ARGS = _args_hist
    LAST_RUNNERS = _runners_hist
    return out_full.astype(np.float32)
